# revision 16
# baseline (speedup 1.0000x reference)
"""Trainium2 Bass kernel for nn_CrossAttention (packed cross-attention).

Math (verified against the jax reference):
  The reference scatters packed rows into dense slots, runs masked dense
  attention over T*N tokens, and gathers pred rows back.  Because q is zero
  in ctx slots, k/v are zero in pred slots, and (pred x pred) pairs are
  masked to -inf, this is exactly: for each batch b, the packed pred rows
  cross-attend to the packed ctx rows of the same batch:

    Q = Xp_b @ Wq ; [K|V] = Xc_b @ Wkv          (Xp_b, Xc_b: [1024, 512])
    out_b = concat_h( softmax(Q_h K_h^T / 8) V_h ) @ Wproj + bproj

  Softmax needs no max-subtraction: |scores| < ~7, exp is safe in fp32.

Sharding: 8 cores = (2 batches) x (4 head-pairs).  Each core computes two
heads of one batch and the partial output projection for those heads
(row-sharded Wproj); the host sums the 4 partials per batch and adds bproj.

Schedule (v2 — rebuilt from the v1 baseline's HW trace):
  - inputs split across BOTH hardware DGE queues (sync + scalar) in
    need-order; weights are pre-tiled on host so every DMA descriptor is a
    contiguous 1-2KB row (256B-element descriptors run at half rate)
  - the activation table is preloaded at t=0 (1.3us) so the first loop exp
    doesn't pay it
  - 7 dummy matmuls on zeroed scratch warm the PE p-state during the DMA
    window (the PE runs 1.2GHz for its first ~3us of busy time, 2.4 after)
  - KT/QT run as x chunks arrive; their PSUM evacuations spread over
    DVE/Pool/Scalar; the first two S^T iterations are emitted BEFORE the
    V projection so the exp stream (the loop bottleneck: 16 x ~1us on
    ScalarE, the only exp-capable engine) starts as early as possible; V
    then fills the PE while exps 0-1 run
  - attention loop: S^T one iteration ahead of exp, PV trails; PSUM:
    s double-buffered [128,1024] (4 banks) + per-head pv [65,1024] (4)
  - tail: Ln(Z row) on ScalarE -> broadcast down 64 partitions via a K=1
    ones-column f32r matmul (no zero-padded ones tile needed) -> exp(-x)
    on the packed [128,512] tile (both heads at once) -> per-head
    normalize-multiplies on DVE/Pool -> heads packed vertically in one
    [128,1024] O^T tile so the output projection is ONE K=128 matmul per
    query tile (8 total instead of 16) -> casts rotate DVE/Pool/Scalar ->
    output DMAs split across both queues
"""

import sys

if "/opt/trn_rl_repo" not in sys.path:
    sys.path.insert(0, "/opt/trn_rl_repo")

import numpy as np

B, T, N, C, H = 2, 8, 256, 512, 8
T_CTX = T // 2
HD = C // H            # 64
SEQ = T_CTX * N        # 1024 packed tokens per batch (q and kv)
NCORE = 8
CT_N = C // 128        # 4 contraction tiles over C
KT_N = SEQ // 128      # 8 key tiles
SCALE = HD ** -0.5

_PROG = None
SPLIT_WAITS = True  # walrus needs it; CoreSim chokes on it


def _build_program():
    import concourse.bass as bass
    import concourse.tile as tile
    from concourse import mybir

    F16 = mybir.dt.float16

    class TrimTailTileContext(tile.TileContext):
        """Skip the second end-of-kernel all-engine barrier: executions of
        the NEFF are serialized by the runtime, and the semaphore clear is
        still ordered after the first barrier on the gpsimd queue."""

        def _drain_and_barrier(self, tick_clock, wait_clock):
            from concourse.vector_clock import ScopedClock

            drain_inst = self.nc.sync.drain()
            wait_clock.add_sem_waits(
                drain_inst.ins, ScopedClock({None: tick_clock.global_clock}))
            self.nc.all_engine_barrier()
            popped = self.nc._tile_sem_poison_stack.pop()
            assert popped is self._sem_poison
            self.nc.clear_and_free_semaphores(
                list(self.sems.allocated().values()))

    nc = bass.Bass("TRN2", target_bir_lowering=False, debug=False,
                   num_devices=NCORE)

    xpT = nc.dram_tensor("xpT", [C, SEQ], F16, kind="ExternalInput").ap()
    xcT = nc.dram_tensor("xcT", [C, SEQ], F16, kind="ExternalInput").ap()
    # weights pre-tiled on host: w*[p, ct*128 + d] = W[ct*128 + p, d]
    wq = nc.dram_tensor("wq", [128, C], F16, kind="ExternalInput").ap()
    wk = nc.dram_tensor("wk", [128, C], F16, kind="ExternalInput").ap()
    wv = nc.dram_tensor("wv", [128, C], F16, kind="ExternalInput").ap()
    wp = nc.dram_tensor("wp", [128, C], F16, kind="ExternalInput").ap()
    out = nc.dram_tensor("out", [SEQ, C], F16, kind="ExternalOutput").ap()

    with TrimTailTileContext(nc) as tc:
        _emit(nc, tc, mybir, xpT, xcT, wq, wk, wv, wp, out)
    if SPLIT_WAITS:
        _split_sync_waits(nc, mybir)
    return nc


def _split_sync_waits(nc, mybir):
    """This container's walrus build has tight per-instruction sync-wait
    limits ("Too many sync wait commands": Matmult holds 1 wait command,
    control-class instructions 2).  Tile freely assigns more.  Rewrite each
    block, moving overflow waits onto same-engine NoOps inserted directly
    before the over-limit instruction (safe: the engine queue executes in
    order, so the waits still complete before the instruction runs)."""
    LIMITS = {}
    DEFAULT = 1
    NOP_W = 1
    n = 0
    for fn in nc.m.functions:
        for bb in fn.blocks:
            insts = bb.instructions
            new = []
            changed = False
            for inst in insts:
                si = inst.sync_info
                waits = list(si.on_wait) if si is not None else []
                limit = LIMITS.get(inst.opcode, DEFAULT)
                if len(waits) > limit:
                    extra = waits[:-limit] if limit else waits
                    keep = waits[-limit:] if limit else []
                    # the end-of-kernel drain carries one wait per logical
                    # processor; spread its nops across engines so they
                    # retire in parallel (the following barrier re-syncs),
                    # instead of ~130ns each serially on the sync sequencer
                    if inst.opcode == "Drain" and len(extra) > 4:
                        engs = [mybir.EngineType.SP, mybir.EngineType.PE,
                                mybir.EngineType.DVE,
                                mybir.EngineType.Activation,
                                mybir.EngineType.Pool]
                    else:
                        engs = [inst.engine]
                    for i in range(0, len(extra), NOP_W):
                        nop = mybir.InstNoOp(
                            name=f"I-waitsplit-{n}", ins=[], outs=[],
                            engine=engs[(i // NOP_W) % len(engs)],
                            sync_info=mybir.SyncInfo(
                                on_wait=extra[i:i + NOP_W], on_update=[]))
                        new.append(nop)
                        n += 1
                    inst.sync_info = mybir.SyncInfo(
                        on_wait=keep, on_update=list(si.on_update))
                    changed = True
                new.append(inst)
            if changed:
                bb.instructions = new


def _emit(nc, tc, mybir, xpT, xcT, wq, wk, wv, wp, out):
    from contextlib import ExitStack

    F32 = mybir.dt.float32
    F16 = mybir.dt.float16
    Exp = mybir.ActivationFunctionType.Exp
    Ln = mybir.ActivationFunctionType.Ln

    with ExitStack() as ctx:
        sb = ctx.enter_context(tc.tile_pool(name="sb", bufs=1))

        # x tensors in 2 chunks of 2 contraction tiles each (fewer DMA
        # issue slots, still chunk-paced consumption)
        xp_sb = [sb.tile([128, 2, SEQ], F16, tag=f"xp{j}", name=f"xp{j}")
                 for j in range(2)]
        xc_sb = [sb.tile([128, 2, SEQ], F16, tag=f"xc{j}", name=f"xc{j}")
                 for j in range(2)]
        wq_sb = sb.tile([128, CT_N, 128], F16, tag="wq")
        wk_sb = sb.tile([128, CT_N, 128], F16, tag="wk")
        wv_sb = sb.tile([128, CT_N, 128], F16, tag="wv")
        wp_sb = sb.tile([128, C], F16, tag="wp")
        qt_p = [sb.tile([128, SEQ], F16, tag=f"qt{h}", name=f"qt{h}")
                for h in range(2)]
        kt_p = [sb.tile([128, SEQ], F16, tag=f"kt{h}", name=f"kt{h}")
                for h in range(2)]
        vones = [sb.tile([128, 4, 130], F16, tag=f"vones{g}", name=f"vones{g}")
                 for g in range(2)]
        z16 = [sb.tile([1, SEQ], F16, tag=f"z{h}", name=f"z{h}")
               for h in range(2)]
        ones1 = sb.tile([1, 128], F16, tag="ones1")
        otn = sb.tile([128, SEQ], F16, tag="otn")
        rz = [sb.tile([64, SEQ], F32, tag=f"rz{h}", name=f"rz{h}")
              for h in range(2)]
        warm = sb.tile([128, 512], F16, tag="warm")
        tpre = sb.tile([1, 8], F16, tag="tpre")
        tpre2 = sb.tile([1, 8], F16, tag="tpre2")
        # deep rotation: the PV consume lags its exp by 4 items
        p_t = [sb.tile([128, SEQ], F16, tag=f"pt{i}", name=f"pt{i}")
               for i in range(8)]
        o16_t = [sb.tile([128, C], F16, tag=f"o16{i}", name=f"o16{i}")
                 for i in range(8)]

        def xc_ap(ct):
            return xc_sb[ct // 2][:, ct % 2, :]

        def xp_ap(ct):
            return xp_sb[ct // 2][:, ct % 2, :]

        # ---- t=0: act-table preload + warmup scratch (gpsimd memsets
        # run earliest; the vector queue idles ~2us longer at startup) ----
        nc.gpsimd.memset(tpre[:], 0.0)
        nc.gpsimd.memset(warm[:], 0.0)
        nc.scalar.activation(out=tpre2[:], in_=tpre[:], func=Exp)

        # ---- input DMAs: need-order, split across both HWDGE queues ----
        def xin(j, dst, srcT, eng):
            eng.dma_start(
                out=dst[j][:],
                in_=srcT[j * 256:(j + 1) * 256, :].rearrange(
                    "(i p) d -> p i d", p=128))

        nc.sync.dma_start(out=wk_sb[:].rearrange("p ct d -> p (ct d)"),
                          in_=wk)
        xin(0, xc_sb, xcT, nc.sync)
        xin(1, xc_sb, xcT, nc.sync)
        nc.sync.dma_start(out=wv_sb[:].rearrange("p ct d -> p (ct d)"),
                          in_=wv)
        nc.scalar.dma_start(out=wq_sb[:].rearrange("p ct d -> p (ct d)"),
                            in_=wq)
        xin(0, xp_sb, xpT, nc.scalar)
        xin(1, xp_sb, xpT, nc.scalar)
        nc.scalar.dma_start(out=wp_sb[:], in_=wp)

        # ---- constant / zero-pad memsets (overlap the DMA window) ----
        nc.vector.memset(ones1[:], 1.0)
        for g in range(2):
            nc.vector.memset(vones[g][:, :, 64:65], 1.0)
            nc.vector.memset(vones[g][:, :, 129:130], 1.0)
        nc.vector.memset(qt_p[0][64:128, :], 0.0)
        nc.vector.memset(qt_p[1][0:64, :], 0.0)
        nc.vector.memset(kt_p[0][64:128, :], 0.0)
        nc.vector.memset(kt_p[1][0:64, :], 0.0)

        # ---- PE p-state warmup during the DMA window ----
        with ExitStack() as wctx:
            warm_pool = wctx.enter_context(
                tc.tile_pool(name="warm_ps", bufs=1, space="PSUM"))
            warm_ps = warm_pool.tile([128, 512], F32, tag="warmps")
            for i in range(8):
                nc.tensor.matmul(out=warm_ps[:], lhsT=warm[:, 0:128],
                                 rhs=warm[:], start=True, stop=True)

        # s pool opened BEFORE qkt so pool release stays stack-ordered
        # (qkt pops first); s(4)+qkt(4) = all 8 banks during the front
        s_pool = ctx.enter_context(
            tc.tile_pool(name="s_ps", bufs=1, space="PSUM"))
        s_t = [s_pool.tile([128, SEQ], F32, tag=f"st{i}", name=f"st{i}")
               for i in range(2)]
        items = [(kt, h) for kt in range(KT_N) for h in range(2)]

        def emit_st(i):
            kt, h = items[i]
            s = s_t[i % 2]
            for nh in range(2):
                nc.tensor.matmul(
                    out=s[:, nh * 512:(nh + 1) * 512],
                    lhsT=kt_p[h][:, kt * 128:(kt + 1) * 128],
                    rhs=qt_p[h][:, nh * 512:(nh + 1) * 512],
                    start=True, stop=True)

        def emit_exp(i):
            nc.scalar.activation(out=p_t[i % 8][:], in_=s_t[i % 2][:],
                                 func=Exp, scale=float(SCALE))

        # ---- KT / QT, merged 2-bank psums, 4 merged evacuations ----
        with ExitStack() as qctx:
            qkt_pool = qctx.enter_context(
                tc.tile_pool(name="qkt_ps", bufs=1, space="PSUM"))
            kt_ps = qkt_pool.tile([128, SEQ], F32, tag="ktps")
            qt_ps = qkt_pool.tile([128, SEQ], F32, tag="qtps")
            # interleave K/Q groups and evacuate each [64,512] piece the
            # moment its group closes so the first S^T fires ~0.7us after
            # the last projection matmul instead of after a 2x1.2us chain
            for nh in range(2):
                o = nh * 512
                for ct in range(CT_N):
                    nc.tensor.matmul(
                        out=kt_ps[:, o:o + 512],
                        lhsT=wk_sb[:, ct, :],
                        rhs=xc_ap(ct)[:, o:o + 512],
                        start=(ct == 0), stop=(ct == CT_N - 1))
                for ct in range(CT_N):
                    nc.tensor.matmul(
                        out=qt_ps[:, o:o + 512],
                        lhsT=wq_sb[:, ct, :],
                        rhs=xp_ap(ct)[:, o:o + 512],
                        start=(ct == 0), stop=(ct == CT_N - 1))
                nc.vector.tensor_copy(out=kt_p[0][0:64, o:o + 512],
                                      in_=kt_ps[0:64, o:o + 512])
                nc.scalar.copy(out=kt_p[1][64:128, o:o + 512],
                               in_=kt_ps[64:128, o:o + 512])
                nc.vector.tensor_copy(out=qt_p[0][0:64, o:o + 512],
                                      in_=qt_ps[0:64, o:o + 512])
                nc.scalar.copy(out=qt_p[1][64:128, o:o + 512],
                               in_=qt_ps[64:128, o:o + 512])

        # ---- S^T chain start + V interleaved (v banks reuse qkt's) ----
        with ExitStack() as vctx:
            v_pool = vctx.enter_context(
                tc.tile_pool(name="v_ps", bufs=1, space="PSUM"))
            v_ps = [v_pool.tile([128, 128], F32, tag=f"vps{i}",
                                name=f"vps{i}") for i in range(2)]

            def emit_v(kt):
                vt = v_ps[kt % 2]
                for ct in range(CT_N):
                    nc.tensor.matmul(
                        out=vt[:],
                        lhsT=xc_ap(ct)[:, kt * 128:(kt + 1) * 128],
                        rhs=wv_sb[:, ct, :],
                        start=(ct == 0), stop=(ct == CT_N - 1))
                dst = vones[kt // 4][:, kt % 4, :].rearrange(
                    "p (g s) -> p g s", g=2)[:, :, 0:64]
                nc.vector.tensor_copy(
                    out=dst, in_=vt[:].rearrange("p (g s) -> p g s", g=2))

            emit_st(0)
            emit_st(1)
            for g in range(4):
                emit_exp(g)
                emit_st(g + 2)
                emit_v(2 * g)
                emit_v(2 * g + 1)

        pv_pool = ctx.enter_context(
            tc.tile_pool(name="pv_ps", bufs=1, space="PSUM"))
        pv = [pv_pool.tile([65, SEQ], F32, tag=f"pv{i}", name=f"pv{i}")
              for i in range(2)]

        def emit_pv(i):
            kt, h = items[i]
            p = p_t[i % 8]
            for nh in range(2):
                nc.tensor.matmul(
                    out=pv[h][:, nh * 512:(nh + 1) * 512],
                    lhsT=vones[kt // 4][:, kt % 4, h * 65:h * 65 + 65],
                    rhs=p[:, nh * 512:(nh + 1) * 512],
                    start=(kt == 0), stop=(kt == KT_N - 1))

        # ---- main loop: exp leads, S^T two ahead, PV lags 4 so it can
        # fill PE slack without stalling the exp stream ----
        for j in range(4, 20):
            if j <= 15:
                emit_exp(j)
                if j + 2 <= 15:
                    emit_st(j + 2)
            emit_pv(j - 4)

        # ---- tail: normalize (packed heads), project, store ----
        # ln Z row [1,1024] on ScalarE; broadcast down the partitions via
        # a K=1 ones-column matmul into a full [128,512] region per head;
        # exp(-x) of the 0:64 half gives 1/Z; DVE multiplies pack the two
        # normalized O^T heads vertically so the output projection is ONE
        # K=128 matmul per query tile.  Tail psum lives in the s_t tiles.
        rbc_ps = [s_t[0][:, 0:512], s_t[0][:, 512:1024]]
        out_slot = [s_t[1][:, 0:512], s_t[1][:, 512:1024],
                    rbc_ps[0], rbc_ps[1]]
        for h in range(2):
            nc.scalar.activation(out=z16[h][0:1, :],
                                 in_=pv[h][64:65, :], func=Ln)
        for nh in range(2):
            o = nh * 512
            for h in range(2):
                nc.tensor.matmul(
                    out=rbc_ps[h], lhsT=ones1[0:1, :],
                    rhs=z16[h][0:1, o:o + 512],
                    start=True, stop=True)
            for h in range(2):
                nc.scalar.activation(out=rz[h][0:64, o:o + 512],
                                     in_=rbc_ps[h][0:64, :], func=Exp,
                                     scale=-1.0)
            nc.vector.tensor_mul(out=otn[0:64, o:o + 512],
                                 in0=pv[0][0:64, o:o + 512],
                                 in1=rz[0][:, o:o + 512])
            nc.vector.tensor_mul(out=otn[64:128, o:o + 512],
                                 in0=pv[1][0:64, o:o + 512],
                                 in1=rz[1][:, o:o + 512])
        # all 8 projections after normalization completes: 4 psum slots
        # (the rbc regions are dead once the exps above have read them),
        # casts alternate Scalar/Vector so the pipeline never waits on a
        # single engine
        for qt in range(8):
            q = qt * 128
            ot = out_slot[qt % 4]
            nc.tensor.matmul(out=ot, lhsT=otn[:, q:q + 128],
                             rhs=wp_sb[:], start=True, stop=True)
            o16 = o16_t[qt]
            if qt % 2 == 0:
                nc.scalar.copy(out=o16[:], in_=ot)
            else:
                nc.vector.tensor_copy(out=o16[:], in_=ot)
            dma = nc.scalar if qt >= 6 else nc.sync
            dma.dma_start(out=out[q:q + 128, :], in_=o16[:])


def _get_program():
    global _PROG
    if _PROG is None:
        _PROG = _build_program()
    return _PROG


def _tile_w(w):
    """[512, 128] head-slice -> [128, 512] with w_t[p, ct*128+d] =
    w[ct*128+p, d] so the DMA reads contiguous 1KB rows."""
    return np.ascontiguousarray(
        w.reshape(CT_N, 128, 128).transpose(1, 0, 2).reshape(128, C))


def _shard_inputs(x_pred, x_ctx, ctx_mask, Wq, Wkv, Wproj):
    """Build the 8 per-core input maps (host-side sharding)."""
    ctx_mask = np.asarray(ctx_mask).astype(bool)
    pidx = np.nonzero(~ctx_mask.reshape(-1))[0]
    cidx = np.nonzero(ctx_mask.reshape(-1))[0]
    pm = [np.where(pidx // T == b)[0] for b in range(B)]
    cm = [np.where(cidx // T == b)[0] for b in range(B)]
    for b in range(B):
        assert len(pm[b]) == T_CTX and len(cm[b]) == T_CTX, (
            "kernel compiled for T_CTX ctx/pred slots per batch row")

    xpT_b, xcT_b = [], []
    for b in range(B):
        Xp = x_pred[pm[b]].reshape(SEQ, C)
        Xc = x_ctx[cm[b]].reshape(SEQ, C)
        xpT_b.append(np.ascontiguousarray(Xp.T).astype(np.float16))
        xcT_b.append(np.ascontiguousarray(Xc.T).astype(np.float16))

    wq16 = Wq.astype(np.float16)
    wk16 = Wkv[:, :C].astype(np.float16)
    wv16 = Wkv[:, C:].astype(np.float16)
    wp16 = Wproj.astype(np.float16)

    in_maps = []
    for c in range(NCORE):
        b, hp = divmod(c, 4)
        hc = hp * 128
        in_maps.append({
            "xpT": xpT_b[b],
            "xcT": xcT_b[b],
            "wq": _tile_w(wq16[:, hc:hc + 128]),
            "wk": _tile_w(wk16[:, hc:hc + 128]),
            "wv": _tile_w(wv16[:, hc:hc + 128]),
            "wp": np.ascontiguousarray(wp16[hc:hc + 128, :]),
        })
    return in_maps, pm


def _unshard_output(results, pm, bproj, dtype):
    full = np.zeros((B * T_CTX, N, C), dtype)
    for b in range(B):
        acc = results[4 * b]["out"].astype(np.float64)
        for j in range(1, 4):
            acc = acc + results[4 * b + j]["out"]
        acc = (acc + bproj).astype(dtype)
        full[pm[b]] = acc.reshape(T_CTX, N, C)
    return full


def run(inputs, trace=False, **kwargs):
    """Run the SPMD kernel; returns (full_output, BassKernelResults)."""
    from concourse.bass_utils import run_bass_kernel_spmd

    nc = _get_program()
    in_maps, pm = _shard_inputs(inputs["x_pred"], inputs["x_ctx"],
                                inputs["ctx_mask"], inputs["Wq"],
                                inputs["Wkv"], inputs["Wproj"])
    res = run_bass_kernel_spmd(nc, in_maps, list(range(NCORE)), trace=trace,
                               **kwargs)
    out = _unshard_output(res.results, pm, np.asarray(inputs["bproj"]),
                          np.asarray(inputs["x_pred"]).dtype)
    return out, res


def kernel(x_pred, x_ctx, ctx_mask, Wq, Wkv, Wproj, bproj):
    out, _ = run(dict(x_pred=np.asarray(x_pred), x_ctx=np.asarray(x_ctx),
                      ctx_mask=np.asarray(ctx_mask), Wq=np.asarray(Wq),
                      Wkv=np.asarray(Wkv), Wproj=np.asarray(Wproj),
                      bproj=np.asarray(bproj)))
    return out
